# revision 1
# baseline (speedup 1.0000x reference)
"""Trainium2 Bass kernel for a dense transformer block (B=4,T=2048,C=1024,H=16,F=4096).

Sharding (8 cores, no collectives): core 2s+p owns sequence s (p=parity).
Even cores take q-chunks (0,3) of 512 tokens, odd cores (1,2); both run ONE
SPMD program with a uniform (8,16) kt-tile structure (extra kt-tiles are
masked to zero via host-supplied multiplicative masks).  Each core computes
ln1 + K,V for its whole sequence, Q only for its 1024 q-tokens.

Attention computes scores pre-transposed (S^T[k,q] = K Q^T) so softmax needs
no on-chip transpose of P; exp is unnormalized (logits are O(1); no max
subtraction) and the denominator comes from a ones-column appended to V.
Normalization is applied post-AV via a DRAM-roundtrip partition broadcast.
"""
import sys, types
import numpy as np
import ml_dtypes

# --- make the NTFF profile hook importable (missing module in this image) ---
def _install_hooks():
    try:
        import antenv
        if "antenv.axon_hooks" not in sys.modules:
            m = types.ModuleType("antenv.axon_hooks")
            m._hook = None
            m.set_axon_ntff_profile_hook = lambda h: setattr(m, "_hook", h)
            m.get_axon_ntff_profile_hook = lambda: m._hook
            sys.modules["antenv.axon_hooks"] = m
            antenv.axon_hooks = m
    except Exception:
        pass
_install_hooks()

import concourse.bass as bass
import concourse.tile as tile
from concourse import mybir, bacc
from concourse.bass_utils import run_bass_kernel_spmd

BF16 = mybir.dt.bfloat16
F32 = mybir.dt.float32
bfloat16 = ml_dtypes.bfloat16

T, C, H, D, F = 2048, 1024, 16, 64, 4096
CH = 512            # q-chunk width
NKT = (8, 16)       # uniform kt-tile counts per (chunkA, chunkB)
P = 128

_cache = {}


def build_nc(apply_ln1w, apply_ln2w, add_bfc2, add_bfc1=True):
    nc = bacc.Bacc()
    x_seq = nc.declare_dram_parameter("x_seq", [T, C], F32, isOutput=False)
    w_attn = nc.declare_dram_parameter("w_attn", [C, 3 * C], BF16, isOutput=False)
    w_proj = nc.declare_dram_parameter("w_proj", [C, C], BF16, isOutput=False)
    w_fc1 = nc.declare_dram_parameter("w_fc1", [C, F], BF16, isOutput=False)
    w_fc2 = nc.declare_dram_parameter("w_fc2", [F, C], BF16, isOutput=False)
    ln1w = nc.declare_dram_parameter("ln1w_rep", [P, C], F32, isOutput=False)
    ln2w = nc.declare_dram_parameter("ln2w_rep", [P, C], F32, isOutput=False)
    bfc1 = nc.declare_dram_parameter("bfc1_col", [P, F // P], F32, isOutput=False)
    bfc2 = nc.declare_dram_parameter("bfc2_rep", [P, C], F32, isOutput=False)
    ident = nc.declare_dram_parameter("ident", [P, P], BF16, isOutput=False)
    maskA = nc.declare_dram_parameter("maskA", [P, NKT[0], CH], BF16, isOutput=False)
    maskB = nc.declare_dram_parameter("maskB", [P, 8, CH], BF16, isOutput=False)
    out = nc.declare_dram_parameter("out", [2 * CH, C], F32, isOutput=True)

    # q-chunk token offsets must be uniform across cores -> host pre-gathers the
    # two q-chunks of x into x_q [1024, C] and k/v use x_seq directly.
    x_q = nc.declare_dram_parameter("x_q", [2 * CH, C], F32, isOutput=False)

    Q_T = 2 * CH           # q tokens per core
    NT, NQ = T // P, Q_T // P
    NC8 = C // P           # 8 contraction tiles

    with tile.TileContext(nc) as tc:
        with tc.tile_pool(name="consts", bufs=1) as consts, \
             tc.tile_pool(name="p_yT", bufs=1) as p_yT, \
             tc.tile_pool(name="dram", bufs=8, space="DRAM") as dram:
            id_sb = consts.tile([P, P], BF16)
            nc.sync.dma_start(out=id_sb[:], in_=ident[:])
            eps_sb = consts.tile([P, 1], F32)
            nc.vector.memset(eps_sb[:], 1e-5)
            ln1w_sb = consts.tile([P, C], F32)
            if apply_ln1w:
                nc.sync.dma_start(out=ln1w_sb[:], in_=ln1w[:])
            ln2w_sb = consts.tile([P, C], F32)
            if apply_ln2w:
                nc.sync.dma_start(out=ln2w_sb[:], in_=ln2w[:])
            bfc1_sb = consts.tile([P, F // P], F32)
            nc.sync.dma_start(out=bfc1_sb[:], in_=bfc1[:])
            bfc2_sb = consts.tile([P, C], F32)
            if add_bfc2:
                nc.sync.dma_start(out=bfc2_sb[:], in_=bfc2[:])

            yT_sb = p_yT.tile([P, NC8, Q_T], BF16)      # [2 heads][hp][q]

            # ============ phase A: ln1 + transpose + QKV ============
            with tc.tile_pool(name="p_kqv", bufs=1) as p_kqv:
                kT_sb = p_kqv.tile([P, NC8, T], BF16)       # [d(2 heads)][hp][t]
                qT_sb = p_kqv.tile([P, NC8, Q_T], BF16)
                v_sb = p_kqv.tile([P, NT, H, D + 1], BF16)  # ones col at [..,64]
                nc.vector.memset(v_sb[:, :, :, D:D + 1], 1.0)

                with tc.tile_pool(name="p_h1T", bufs=1) as p_h1T, \
                     tc.tile_pool(name="p_lnt", bufs=2) as p_lnt, \
                     tc.tile_pool(name="p_w8", bufs=2) as p_w8, \
                     tc.tile_pool(name="ps_tr", bufs=2, space="PSUM") as ps_tr, \
                     tc.tile_pool(name="ps_mm", bufs=6, space="PSUM") as ps_mm:
                    h1T_sb = p_h1T.tile([P, NC8, T], BF16)
                    h1qT_sb = p_h1T.tile([P, NC8, Q_T], BF16)

                    def layernorm_tile(x_t, w_sb, apply_w, out_bf):
                        st = p_lnt.tile([P, 2, 6], F32, tag="lnst")
                        xr = x_t.rearrange("p (s d) -> p s d", s=2)
                        for s in range(2):
                            nc.vector.bn_stats(out=st[:, s, :], in_=xr[:, s, :])
                        mv = p_lnt.tile([P, 2], F32, tag="lnmv")
                        nc.vector.bn_aggr(out=mv[:], in_=st[:])
                        rstd = p_lnt.tile([P, 1], F32, tag="lnrs")
                        nc.scalar.activation(out=rstd[:], in_=mv[:, 1:2],
                                             func=mybir.ActivationFunctionType.Sqrt,
                                             bias=eps_sb[:], scale=1.0)
                        nc.vector.reciprocal(out=rstd[:], in_=rstd[:])
                        if apply_w:
                            tmp = p_lnt.tile([P, C], F32, tag="lntmp")
                            nc.vector.tensor_scalar(
                                out=tmp[:], in0=x_t[:], scalar1=mv[:, 0:1],
                                scalar2=rstd[:], op0=mybir.AluOpType.subtract,
                                op1=mybir.AluOpType.mult)
                            nc.vector.tensor_mul(out=out_bf[:], in0=tmp[:], in1=w_sb[:])
                        else:
                            nc.vector.tensor_scalar(
                                out=out_bf[:], in0=x_t[:], scalar1=mv[:, 0:1],
                                scalar2=rstd[:], op0=mybir.AluOpType.subtract,
                                op1=mybir.AluOpType.mult)

                    # ln1 over the full sequence -> h1T; q tokens (host-gathered
                    # x_q, core-dependent positions) get their own uniform pass -> h1qT.
                    for tt in range(NT + NQ):
                        x_t = p_lnt.tile([P, C], F32, tag="xt")
                        if tt < NT:
                            nc.gpsimd.dma_start(out=x_t[:], in_=x_seq[tt * P:(tt + 1) * P, :])
                        else:
                            nc.gpsimd.dma_start(out=x_t[:], in_=x_q[(tt - NT) * P:(tt - NT + 1) * P, :])
                        h1_t = p_lnt.tile([P, C], BF16, tag="h1t")
                        layernorm_tile(x_t, ln1w_sb, apply_ln1w, h1_t)
                        for co in range(NC8):
                            tp = ps_tr.tile([P, P], BF16)
                            nc.tensor.transpose(tp[:], h1_t[:, co * P:(co + 1) * P], id_sb[:])
                            dst = (h1T_sb[:, co, tt * P:(tt + 1) * P] if tt < NT
                                   else h1qT_sb[:, co, (tt - NT) * P:(tt - NT + 1) * P])
                            if co % 2 == 0:
                                nc.vector.tensor_copy(out=dst, in_=tp[:])
                            else:
                                nc.scalar.copy(out=dst, in_=tp[:])

                    # K and Q: lhsT = w tile, rhs = h1T  -> [feat, tok]
                    # fgroups: 0,1 -> q cols [0:1024]; 2,3 -> k cols [1024:2048]; 4,5 -> v
                    for fg in (2, 3, 4, 5, 0, 1):
                        w8 = p_w8.tile([P, NC8, CH], BF16, tag="w8")
                        nc.gpsimd.dma_start(
                            out=w8[:],
                            in_=w_attn.rearrange("(co p) f -> p co f", p=P)[:, :, fg * CH:(fg + 1) * CH])
                        if fg in (0, 1):        # Q
                            for ft in range(4):
                                for qc in range(2):
                                    pm = ps_mm.tile([P, CH], F32)
                                    for ct in range(NC8):
                                        nc.tensor.matmul(
                                            pm[:], w8[:, ct, ft * P:(ft + 1) * P],
                                            h1qT_sb[:, ct, qc * CH:(qc + 1) * CH],
                                            start=(ct == 0), stop=(ct == NC8 - 1))
                                    nc.vector.tensor_copy(
                                        out=qT_sb[:, fg * 4 + ft, qc * CH:(qc + 1) * CH], in_=pm[:])
                        elif fg in (2, 3):      # K
                            for ft in range(4):
                                for tcx in range(T // CH):
                                    pm = ps_mm.tile([P, CH], F32)
                                    for ct in range(NC8):
                                        nc.tensor.matmul(
                                            pm[:], w8[:, ct, ft * P:(ft + 1) * P],
                                            h1T_sb[:, ct, tcx * CH:(tcx + 1) * CH],
                                            start=(ct == 0), stop=(ct == NC8 - 1))
                                    nc.vector.tensor_copy(
                                        out=kT_sb[:, (fg - 2) * 4 + ft, tcx * CH:(tcx + 1) * CH], in_=pm[:])
                        else:                   # V natural: lhsT = h1T tile, rhs = w
                            for tt in range(NT):
                                pm = ps_mm.tile([P, CH], F32)
                                for ct in range(NC8):
                                    nc.tensor.matmul(
                                        pm[:], h1T_sb[:, ct, tt * P:(tt + 1) * P],
                                        w8[:, ct, :],
                                        start=(ct == 0), stop=(ct == NC8 - 1))
                                h0 = (fg - 4) * 8
                                nc.vector.tensor_copy(
                                    out=v_sb[:, tt, h0:h0 + 8, 0:D],
                                    in_=pm.rearrange("p (h d) -> p h d", d=D))

                # ============ attention ============
                with tc.tile_pool(name="p_att", bufs=1) as p_att, \
                     tc.tile_pool(name="p_pt", bufs=2) as p_pt, \
                     tc.tile_pool(name="p_nrm", bufs=4) as p_nrm, \
                     tc.tile_pool(name="ps_st", bufs=2, space="PSUM") as ps_st, \
                     tc.tile_pool(name="ps_av", bufs=3, space="PSUM") as ps_av:
                    mA_sb = p_att.tile([P, NKT[0], CH], BF16)
                    nc.sync.dma_start(out=mA_sb[:], in_=maskA[:])
                    mB_sb = p_att.tile([P, 8, CH], BF16)
                    nc.sync.dma_start(out=mB_sb[:], in_=maskB[:])

                    for ci in range(2):
                        n_kt = NKT[ci]
                        for h in range(H):
                            hp, h2 = h // 2, h % 2
                            r0 = h2 * D
                            pt = p_pt.tile([P, NKT[1], CH], BF16, tag="pt")
                            for ktp in range(n_kt // 2):
                                sp = ps_st.tile([P, 2 * CH], F32)
                                for k2 in range(2):
                                    kt = 2 * ktp + k2
                                    nc.tensor.matmul(
                                        sp[:, k2 * CH:(k2 + 1) * CH],
                                        kT_sb[r0:r0 + D, hp, kt * P:(kt + 1) * P],
                                        qT_sb[r0:r0 + D, hp, ci * CH:(ci + 1) * CH],
                                        start=True, stop=True)
                                nc.scalar.activation(
                                    pt.rearrange("p k a -> p (k a)")[:, 2 * ktp * CH:(2 * ktp + 2) * CH],
                                    sp[:],
                                    mybir.ActivationFunctionType.Exp, scale=0.125)
                            if ci == 0:
                                nc.vector.tensor_mul(out=pt[:, 0:8, :], in0=pt[:, 0:8, :], in1=mA_sb[:])
                            else:
                                nc.vector.tensor_mul(out=pt[:, 8:16, :], in0=pt[:, 8:16, :], in1=mB_sb[:])
                            ap = ps_av.tile([D + 1, CH], F32)
                            for kt in range(n_kt):
                                nc.tensor.matmul(
                                    ap[:], v_sb[:, kt, h, :], pt[:, kt, :],
                                    start=(kt == 0), stop=(kt == n_kt - 1))
                            nrm = p_nrm.tile([P, CH], F32, tag="nrm")
                            nc.vector.reciprocal(out=nrm[D:D + 1, :], in_=ap[D:D + 1, :])
                            dsc = dram.tile([1, CH], F32)
                            nc.sync.dma_start(out=dsc[:], in_=nrm[D:D + 1, :])
                            nc.sync.dma_start(
                                out=nrm[0:D, :],
                                in_=bass.AP(tensor=dsc.tensor, offset=dsc.offset,
                                            ap=[[0, D]] + [list(a) for a in dsc.ap]))
                            if h2 == 0:
                                nc.vector.tensor_mul(
                                    out=yT_sb[0:D, hp, ci * CH:(ci + 1) * CH],
                                    in0=ap[0:D, :], in1=nrm[0:D, :])
                            else:
                                ytmp = p_nrm.tile([D, CH], BF16, tag="ytmp")
                                nc.vector.tensor_mul(out=ytmp[:], in0=ap[0:D, :], in1=nrm[0:D, :])
                                nc.sync.dma_start(
                                    out=yT_sb[D:P, hp, ci * CH:(ci + 1) * CH], in_=ytmp[:])

            # ============ proj + residual ============
            with tc.tile_pool(name="p_xo", bufs=1) as p_xo:
                xo_sb = p_xo.tile([P, NQ, C], F32)          # attn residual out
                with tc.tile_pool(name="p_prj", bufs=1) as p_prj, \
                     tc.tile_pool(name="p_prt", bufs=3) as p_prt, \
                     tc.tile_pool(name="ps_pr", bufs=4, space="PSUM") as ps_pr:
                    wp_sb = p_prj.tile([P, NC8, C], BF16)
                    nc.sync.dma_start(out=wp_sb[:], in_=w_proj.rearrange("(co p) f -> p co f", p=P))
                    for qt in range(NQ):
                        x_t = p_prt.tile([P, C], F32, tag="xr")
                        nc.sync.dma_start(out=x_t[:], in_=x_q[qt * P:(qt + 1) * P, :])
                        for cc in range(2):
                            pm = ps_pr.tile([P, CH], F32)
                            for hp in range(NC8):
                                nc.tensor.matmul(
                                    pm[:], yT_sb[:, hp, qt * P:(qt + 1) * P],
                                    wp_sb[:, hp, cc * CH:(cc + 1) * CH],
                                    start=(hp == 0), stop=(hp == NC8 - 1))
                            nc.vector.tensor_add(
                                out=xo_sb[:, qt, cc * CH:(cc + 1) * CH],
                                in0=pm[:], in1=x_t[:, cc * CH:(cc + 1) * CH])

                # ============ ln2 + transpose + fc1 ============
                with tc.tile_pool(name="p_a1", bufs=1) as p_a1:
                    a1_sb = p_a1.tile([P, F // P, Q_T], BF16)
                    with tc.tile_pool(name="p_h2T", bufs=1) as p_h2T, \
                         tc.tile_pool(name="p_ln2", bufs=3) as p_ln2, \
                         tc.tile_pool(name="p_w1", bufs=2) as p_w1, \
                         tc.tile_pool(name="ps_t2", bufs=2, space="PSUM") as ps_t2, \
                         tc.tile_pool(name="ps_f1", bufs=6, space="PSUM") as ps_f1:
                        h2T_sb = p_h2T.tile([P, NC8, Q_T], BF16)
                        for qt in range(NQ):
                            h2_t = p_ln2.tile([P, C], BF16, tag="h2t")
                            st = p_ln2.tile([P, 2, 6], F32, tag="l2st")
                            xr = xo_sb[:, qt, :].rearrange("p (s d) -> p s d", s=2)
                            for s in range(2):
                                nc.vector.bn_stats(out=st[:, s, :], in_=xr[:, s, :])
                            mv = p_ln2.tile([P, 2], F32, tag="l2mv")
                            nc.vector.bn_aggr(out=mv[:], in_=st[:])
                            rstd = p_ln2.tile([P, 1], F32, tag="l2rs")
                            nc.scalar.activation(out=rstd[:], in_=mv[:, 1:2],
                                                 func=mybir.ActivationFunctionType.Sqrt,
                                                 bias=eps_sb[:], scale=1.0)
                            nc.vector.reciprocal(out=rstd[:], in_=rstd[:])
                            if apply_ln2w:
                                tmp = p_ln2.tile([P, C], F32, tag="l2tmp")
                                nc.vector.tensor_scalar(
                                    out=tmp[:], in0=xo_sb[:, qt, :], scalar1=mv[:, 0:1],
                                    scalar2=rstd[:], op0=mybir.AluOpType.subtract,
                                    op1=mybir.AluOpType.mult)
                                nc.vector.tensor_mul(out=h2_t[:], in0=tmp[:], in1=ln2w_sb[:])
                            else:
                                nc.vector.tensor_scalar(
                                    out=h2_t[:], in0=xo_sb[:, qt, :], scalar1=mv[:, 0:1],
                                    scalar2=rstd[:], op0=mybir.AluOpType.subtract,
                                    op1=mybir.AluOpType.mult)
                            for co in range(NC8):
                                tp = ps_t2.tile([P, P], BF16)
                                nc.tensor.transpose(tp[:], h2_t[:, co * P:(co + 1) * P], id_sb[:])
                                if co % 2 == 0:
                                    nc.vector.tensor_copy(out=h2T_sb[:, co, qt * P:(qt + 1) * P], in_=tp[:])
                                else:
                                    nc.scalar.copy(out=h2T_sb[:, co, qt * P:(qt + 1) * P], in_=tp[:])

                        # fc1 in four F-quarters; relu+bias on evict
                        FQ = F // 4
                        for fh in range(4):
                            w1 = p_w1.tile([P, NC8, FQ], BF16, tag="w1")
                            nc.gpsimd.dma_start(
                                out=w1[:],
                                in_=w_fc1.rearrange("(co p) f -> p co f", p=P)[:, :, fh * FQ:(fh + 1) * FQ])
                            for ft in range(FQ // P):
                                fg = fh * (FQ // P) + ft
                                for qc in range(2):
                                    pm = ps_f1.tile([P, CH], F32)
                                    for ct in range(NC8):
                                        nc.tensor.matmul(
                                            pm[:], w1[:, ct, ft * P:(ft + 1) * P],
                                            h2T_sb[:, ct, qc * CH:(qc + 1) * CH],
                                            start=(ct == 0), stop=(ct == NC8 - 1))
                                    if add_bfc1:
                                        nc.scalar.activation(
                                            a1_sb[:, fg, qc * CH:(qc + 1) * CH], pm[:],
                                            mybir.ActivationFunctionType.Relu,
                                            bias=bfc1_sb[:, fg:fg + 1])
                                    else:
                                        nc.vector.tensor_scalar_max(
                                            out=a1_sb[:, fg, qc * CH:(qc + 1) * CH],
                                            in0=pm[:], scalar1=0.0)

                    # ============ fc2 + residual (two F-halves, o_t accumulates) ============
                    with tc.tile_pool(name="p_w2", bufs=2) as p_w2, \
                         tc.tile_pool(name="p_ot", bufs=1) as p_ot, \
                         tc.tile_pool(name="ps_f2", bufs=6, space="PSUM") as ps_f2:
                        o_acc = p_ot.tile([P, NQ, C], F32)
                        FH = F // P // 4   # 8 f-tiles per quarter
                        for fh in range(4):
                            w2 = p_w2.tile([P, FH, C], BF16, tag="w2")
                            nc.sync.dma_start(
                                out=w2[:],
                                in_=w_fc2.rearrange("(fo p) c -> p fo c", p=P)[:, fh * FH:(fh + 1) * FH, :])
                            for qt in range(NQ):
                                for cc in range(2):
                                    pm = ps_f2.tile([P, CH], F32)
                                    for ft in range(FH):
                                        nc.tensor.matmul(
                                            pm[:], a1_sb[:, fh * FH + ft, qt * P:(qt + 1) * P],
                                            w2[:, ft, cc * CH:(cc + 1) * CH],
                                            start=(ft == 0), stop=(ft == FH - 1))
                                    if fh == 0:
                                        nc.vector.tensor_add(
                                            out=o_acc[:, qt, cc * CH:(cc + 1) * CH], in0=pm[:],
                                            in1=xo_sb[:, qt, cc * CH:(cc + 1) * CH])
                                    else:
                                        nc.vector.tensor_add(
                                            out=o_acc[:, qt, cc * CH:(cc + 1) * CH], in0=pm[:],
                                            in1=o_acc[:, qt, cc * CH:(cc + 1) * CH])
                        for qt in range(NQ):
                            if add_bfc2:
                                nc.vector.tensor_add(out=o_acc[:, qt, :], in0=o_acc[:, qt, :], in1=bfc2_sb[:])
                            nc.sync.dma_start(out=out[qt * P:(qt + 1) * P, :], in_=o_acc[:, qt, :])
    nc.compile()
    return nc


def _diag_masks():
    # d(j)[p, f] = 1 if f >= 128*j + p  (within one 512 q-chunk)
    f = np.arange(CH)[None, :]
    p = np.arange(P)[:, None]
    d = [(f >= (P * j + p)).astype(np.float32) for j in range(4)]
    one = np.ones((P, CH), np.float32)
    zero = np.zeros((P, CH), np.float32)
    # chunkA blocks 0..7: even core (chunk0): d0..d3, 0,0,0,0 ; odd (chunk1): 1*4, d0..d3
    mA_even = np.stack(d + [zero] * 4, 1)
    mA_odd = np.stack([one] * 4 + d, 1)
    # chunkB blocks 8..15: even (chunk3): 1*4 then d0..d3 ; odd (chunk2): d0..d3 then 0*4
    mB_even = np.stack([one] * 4 + d, 1)
    mB_odd = np.stack(d + [zero] * 4, 1)
    bf = lambda a: np.ascontiguousarray(a).astype(bfloat16)
    return (bf(mA_even), bf(mB_even)), (bf(mA_odd), bf(mB_odd))


def kernel(x, ln1_w, w_attn, w_proj, ln2_w, w_fc1, b_fc1, w_fc2, b_fc2):
    x = np.asarray(x, np.float32)
    B = x.shape[0]
    apply_ln1w = not np.allclose(ln1_w, 1.0)
    apply_ln2w = not np.allclose(ln2_w, 1.0)
    add_bfc2 = not np.allclose(b_fc2, 0.0)
    add_bfc1 = not np.allclose(b_fc1, 0.0)
    key = (apply_ln1w, apply_ln2w, add_bfc2, add_bfc1)
    if key not in _cache:
        _cache[key] = build_nc(*key)
    nc = _cache[key]

    bf = lambda a: np.ascontiguousarray(np.asarray(a, np.float32)).astype(bfloat16)
    shared = dict(
        w_attn=bf(w_attn), w_proj=bf(w_proj), w_fc1=bf(w_fc1), w_fc2=bf(w_fc2),
        ln1w_rep=np.tile(np.asarray(ln1_w, np.float32)[None, :], (P, 1)),
        ln2w_rep=np.tile(np.asarray(ln2_w, np.float32)[None, :], (P, 1)),
        bfc1_col=np.ascontiguousarray(np.asarray(b_fc1, np.float32).reshape(F // P, P).T),
        bfc2_rep=np.tile(np.asarray(b_fc2, np.float32)[None, :], (P, 1)),
        ident=np.eye(P, dtype=np.float32).astype(bfloat16),
    )
    (mAe, mBe), (mAo, mBo) = _diag_masks()
    chunks = ((0, 3), (1, 2))   # parity -> q-chunk ids
    in_maps = []
    for core in range(2 * B):
        s, par = core // 2, core % 2
        c0, c1 = chunks[par]
        xq = np.concatenate([x[s, c0 * CH:(c0 + 1) * CH], x[s, c1 * CH:(c1 + 1) * CH]], 0)
        in_maps.append(dict(
            shared,
            x_seq=np.ascontiguousarray(x[s]),
            x_q=np.ascontiguousarray(xq),
            maskA=mAe if par == 0 else mAo,
            maskB=mBe if par == 0 else mBo,
        ))
    res = run_bass_kernel_spmd(nc, in_maps, list(range(2 * B)))
    out = np.empty_like(x)
    for core in range(2 * B):
        s, par = core // 2, core % 2
        c0, c1 = chunks[par]
        r = res.results[core]["out"]
        out[s, c0 * CH:(c0 + 1) * CH] = r[0:CH]
        out[s, c1 * CH:(c1 + 1) * CH] = r[CH:2 * CH]
    return out

